# revision 26
# baseline (speedup 1.0000x reference)
"""LocalAttentionDraftLayer TRN2 Bass kernel (v2, bf16 + fused weights).

Sharding: sequence-parallel over B*S across 8 cores (each core owns a
contiguous 1024-token chunk of one batch row plus a 32-token halo of
preceding tokens, zero-padded at sequence start). Attention is strictly
local (window 32, causal), so the halo is materialized host-side and no
collectives are needed.

Key algebraic restructurings vs the straightforward layer (all host-side
weight folds, validated to rel err 4.6e-3 in numpy with bf16 operands):

  scores = (x Wq^T)(x Wk^T)^T / sqrt(H) = (x A) x^T  with A = Wq^T Wk / sqrt(H)
    -> the K projection disappears; keys are just x^T (already resident).
  attn_out = P (x Wv^T) Wo^T = (P x) Wvo^T          with Wvo = Wo Wv
    -> the V projection disappears; PV runs directly on x, and the old
       Wo-sized GEMM becomes the only post-attention projection.

Everything on-chip lives in "transposed land" ([feature, token]); all
matmul operands are bf16 (cost model: 1 cycle/row at any N, PSUM f32),
roughly 247K PE cycles/core vs 411K for the fp32r baseline.

Per core:
  qt[h,q]  = A^T x^T                      (q~ = xA, includes 1/sqrt(H))
  per 128-query block b (keys = 160-wide slice of x^T starting at b*128):
    sc = mask_preload + q~ . x            (PSUM-preloaded additive mask,
                                           accumulate with start=False)
    softmax without max-subtraction (scores are O(5); exp is safe):
    pexp = exp(sc) with accum -> rsum; rcp = 1/rsum (DVE);
    pn = pexp * rcp -> bf16 (Act copy-with-scale)
    P^T via two PE transposes (128 + 32 key rows) -> ptg (SBUF)
  y[h,q]   = x_nat^T P^T  per (h-chunk, 512-query half) PSUM group
  attnT    = Wvo y; draftT = attnT + x^T; LN stats via ones-matmul;
  rstd broadcast via K=1 matmul; mean correction as rank-1 matmul folded
  into MLP1 (ln_w folded into W1, ln_b folded into the gelu bias).
  h1 = gelu(W1w drs + nw1s*(mu*rstd) + b1c); outT = W2 h1 + b2 + draftT
Host transposes outT back and stitches the 8 chunks.
"""

import sys

sys.path.insert(0, "/opt/trn_rl_repo")

from contextlib import ExitStack

import numpy as np
import ml_dtypes

import concourse.bacc as bacc
import concourse.tile as tile
from concourse import mybir
from concourse.bass_utils import run_bass_kernel_spmd

B, S, H = 2, 4096, 1024
WIN = 32
N_CORES = 8
SL = S // 4            # 1024 tokens per core
XW = SL + WIN          # 1056 = halo + chunk
NB = SL // 128         # 8 query blocks
KWIN = 160             # keys per query block (128-aligned window cover)

F32 = mybir.dt.float32
BF16 = mybir.dt.bfloat16
OP = mybir.AluOpType
AF = mybir.ActivationFunctionType

_CACHE = {}
DEBUG_TAPS = False


def _build():
    nc = bacc.Bacc("TRN2", target_bir_lowering=False, debug=False,
                   num_devices=N_CORES)

    def din(name, shape, dt=BF16):
        return nc.dram_tensor(name, shape, dt, kind="ExternalInput").ap()

    xT_d = din("xT", [H, XW])
    xn_d = din("xn", [9 * 128, H])
    a_d = din("a", [H, H])
    wvo_d = din("wvo", [H, H])
    w1_d = din("w1", [H, 512])
    w2_d = din("w2", [512, H])
    cb_d = din("cb", [128, 769])          # ident|ones_c|ones_r(row0)|nw1s(row0)
    cf_d = din("cf", [128, 333], F32)     # m0|mR|b1c|b2c|eps
    cm_d = din("cm", [128, 2 * KWIN])     # bf16 masks: m0|mR
    outT = nc.dram_tensor("outT", [H, SL], BF16, kind="ExternalOutput").ap()
    taps = {}
    if DEBUG_TAPS:
        for nm, sh, dt in [("qt_d", [128, 8192], BF16),
                           ("pn0_d", [128, 160], BF16),
                           ("ptg_d", [128, 2048], BF16),
                           ("ysb_d", [128, 8192], BF16),
                           ("draft_d", [128, 8192], BF16),
                           ("drs_d", [128, 8192], BF16),
                           ("h1_d", [128, 4096], BF16),
                           ("stat_d", [1, 1024], F32),
                           ("statb_d", [1, 2048], BF16)]:
            taps[nm] = nc.dram_tensor(nm, sh, dt, kind="ExternalOutput").ap()

    with tile.TileContext(nc) as tc, ExitStack() as ctx:
        sb = ctx.enter_context(tc.tile_pool(name="sb", bufs=1))
        sx = ctx.enter_context(tc.tile_pool(name="sx", bufs=2))
        pp = ctx.enter_context(tc.tile_pool(name="pp", bufs=2, space="PSUM"))
        sp = ctx.enter_context(tc.tile_pool(name="sp", bufs=4, space="PSUM"))
        pt = ctx.enter_context(tc.tile_pool(name="pt", bufs=2, space="PSUM"))

        # ---- SBUF tiles ----
        xt = sb.tile([128, 8 * XW], BF16, tag="xt")
        xn = sb.tile([128, 9 * 1024], BF16, tag="xn")
        qt = sb.tile([128, 8 * 1024], BF16, tag="qt")
        ptg = sb.tile([128, 8 * 256], BF16, tag="ptg")
        ysb = sb.tile([128, 8 * 1024], BF16, tag="ysb")
        draft = sb.tile([128, 8 * 1024], BF16, tag="draft")
        drs = sb.tile([128, 8 * 1024], BF16, tag="drs")
        h1 = sb.tile([128, 4 * 1024], BF16, tag="h1")
        statr = sb.tile([1, 1024], F32, tag="statr")
        statb = sb.tile([1, 2048], BF16, tag="statb")
        cb = sb.tile([128, 769], BF16, tag="cb")
        cf = sb.tile([128, 333], F32, tag="cf")
        cm = sb.tile([128, 2 * KWIN], BF16, tag="cm")
        aq = [sb.tile([128, 2048], BF16, tag=f"a{i}", name=f"aq{i}")
              for i in range(4)]
        wvq = [sb.tile([128, 2048], BF16, tag=f"wv{i}", name=f"wvq{i}")
               for i in range(4)]
        w1q = [sb.tile([128, 2048], BF16, tag=f"w1{i}", name=f"w1q{i}")
               for i in range(2)]
        w2h = [sb.tile([128, 2048], BF16, tag=f"w2{i}", name=f"w2h{i}")
               for i in range(2)]

        identb = cb[:, 0:128]
        ones_c = cb[:, 128:129]
        ones_r = cb[0:1, 129:257]
        nw1s = cb[0:1, 257:769]
        b1c = cf[:, 320:324]
        b2c = cf[:, 324:332]
        eps_t = cf[0:1, 332:333]

        # ---- DMAs, in deadline order ----
        def dma_quarter(dst, src, i, ncols, nchunks):
            nc.sync.dma_start(
                dst[:, :].rearrange("p (c j) -> p c j", c=nchunks),
                src.rearrange("(c p) h -> p c h", p=128)
                [:, :, i * ncols:(i + 1) * ncols])

        xtv = xt[:, :].rearrange("p (c w) -> p c w", c=8)
        xTv = xT_d.rearrange("(c p) w -> p c w", p=128)
        dma_quarter(aq[0], a_d, 0, 256, 8)
        nc.sync.dma_start(xtv[:, :, WIN:WIN + 512], xTv[:, :, WIN:WIN + 512])
        nc.sync.dma_start(cb, cb_d)
        nc.sync.dma_start(cf, cf_d)
        nc.sync.dma_start(cm, cm_d)
        for i in range(1, 4):
            dma_quarter(aq[i], a_d, i, 256, 8)
        nc.sync.dma_start(xtv[:, :, WIN + 512:XW], xTv[:, :, WIN + 512:XW])
        nc.sync.dma_start(xtv[:, :, 0:WIN], xTv[:, :, 0:WIN])
        nc.sync.dma_start(xn[:, :].rearrange("p (c h) -> p c h", c=9),
                          xn_d.rearrange("(c p) h -> p c h", p=128))
        for i in range(4):
            dma_quarter(wvq[i], wvo_d, i, 256, 8)
        for i in range(2):
            dma_quarter(w1q[i], w1_d, i, 256, 8)
        for i in range(2):
            dma_quarter(w2h[i], w2_d, i, 512, 4)

        # ---- PE warmup: junk transposes ramp the PE p-state while the
        # first weight/activation DMAs are in flight (no data deps).
        junk_sb = sx.tile([128, 128], BF16, tag="junk", bufs=1)
        nc.vector.memset(junk_sb[:, :], 0.0)
        junk_f = sx.tile([1, 1], F32, tag="junkf", bufs=1)
        # preload the Exp act-table set while the PE warms up
        nc.scalar.activation(junk_f[:, :], junk_sb[0:1, 0:1], AF.Exp)
        junk_ps = sp.tile([128, 128], BF16, tag="sp", name="junk_ps")
        for _ in range(80):
            nc.tensor.transpose(junk_ps[:, 0:128], junk_sb[:, :],
                                junk_sb[:, :])

        # ---- Phase 1: qt = (x A)^T, layout [h-chunk][128, q] ----
        for qn in range(2):
            for i in range(4):
                for oc in (2 * i, 2 * i + 1):
                    ppt = pp.tile([128, 512], F32, tag="pp",
                                  name=f"q_{oc}_{qn}")
                    for kc in range(8):
                        nc.tensor.matmul(
                            ppt[:, :],
                            aq[i][:, kc * 256 + (oc % 2) * 128:
                                  kc * 256 + (oc % 2) * 128 + 128],
                            xt[:, kc * XW + WIN + qn * 512:
                               kc * XW + WIN + (qn + 1) * 512],
                            start=(kc == 0), stop=(kc == 7))
                    nc.any.tensor_copy(
                        qt[:, oc * 1024 + qn * 512:oc * 1024 + (qn + 1) * 512],
                        ppt[:, :])
        if DEBUG_TAPS:
            nc.sync.dma_start(taps["qt_d"], qt[:, :])

        # ---- Phase 2: local attention ----
        # Per 128-query block b, keys live at x^T cols [b*128, b*128+160);
        # the additive band mask (preloaded into PSUM) zeroes the rest.
        def sc_block(b):
            sct = sp.tile([128, KWIN], F32, tag="sp", name=f"sc{b}")
            msl = cm[:, 0:KWIN] if b == 0 else cm[:, KWIN:2 * KWIN]
            nc.tensor.matmul(sct[:, :], identb, msl, start=True, stop=False)
            for kc in range(8):
                nc.tensor.matmul(
                    sct[:, :],
                    qt[:, kc * 1024 + b * 128:kc * 1024 + (b + 1) * 128],
                    xt[:, kc * XW + b * 128:kc * XW + b * 128 + KWIN],
                    start=False, stop=(kc == 7))
            return sct

        def softmax_block(b, sct):
            pexp = sx.tile([128, KWIN], F32, tag="pe", name=f"pe{b}")
            rsum = sx.tile([128, 1], F32, tag="rs", name=f"rs{b}")
            nc.scalar.activation(pexp[:, :], sct[:, :], AF.Exp,
                                 accum_out=rsum[:, 0:1])
            rcp = sx.tile([128, 1], F32, tag="rc", name=f"rc{b}")
            nc.vector.reciprocal(rcp[:, :], rsum[:, :])
            pn = sx.tile([128, KWIN], BF16, tag="pn", name=f"pn{b}")
            nc.gpsimd.tensor_scalar_mul(pn[:, :], pexp[:, :], rcp[:, 0:1])
            return pn

        def pt_block(b, pn):
            ptA = pt.tile([128, 128], BF16, tag="pt", name=f"ptA{b}")
            nc.tensor.transpose(ptA[:, 0:128], pn[:, 0:128], identb)
            nc.any.tensor_copy(ptg[:, b * 256:b * 256 + 128], ptA[:, 0:128])
            ptB = pt.tile([128, 128], BF16, tag="pt", name=f"ptB{b}")
            nc.tensor.transpose(ptB[0:32, 0:128], pn[:, 128:KWIN], identb)
            nc.any.tensor_copy(ptg[0:32, b * 256 + 128:b * 256 + 256],
                               ptB[0:32, 0:128])

        def y_group(oc, half):
            ypt = pp.tile([128, 512], F32, tag="pp", name=f"y{oc}_{half}")
            for j, bb in enumerate(range(half * 4, half * 4 + 4)):
                nc.tensor.matmul(
                    ypt[:, j * 128:(j + 1) * 128],
                    xn[:, bb * 1024 + oc * 128:bb * 1024 + oc * 128 + 128],
                    ptg[:, bb * 256:bb * 256 + 128],
                    start=(j == 0), stop=False)
            for j, bb in enumerate(range(half * 4, half * 4 + 4)):
                nc.tensor.matmul(
                    ypt[:, j * 128:(j + 1) * 128],
                    xn[0:32, (bb + 1) * 1024 + oc * 128:
                       (bb + 1) * 1024 + oc * 128 + 128],
                    ptg[0:32, bb * 256 + 128:bb * 256 + 256],
                    start=False, stop=(j == 3))
            base = oc * 1024 + half * 512
            nc.any.tensor_copy(ysb[:, base:base + 256], ypt[:, 0:256])
            nc.any.tensor_copy(ysb[:, base + 256:base + 512], ypt[:, 256:512])

        scts = {b: sc_block(b) for b in range(4)}
        for b in range(NB):
            pn = softmax_block(b, scts[b])
            if DEBUG_TAPS and b == 0:
                nc.sync.dma_start(taps["pn0_d"], pn[:, :])
            if b >= 4:
                y_group(2 * (b - 4), 0)
                y_group(2 * (b - 4) + 1, 0)
            if b + 4 < NB:
                scts[b + 4] = sc_block(b + 4)
            pt_block(b, pn)
        for oc in range(8):
            y_group(oc, 1)

        if DEBUG_TAPS:
            nc.sync.dma_start(taps["ptg_d"], ptg[:, :])
            nc.sync.dma_start(taps["ysb_d"], ysb[:, :])

        # ---- Phase 3: draftT = Wvo y + x^T; LN stats; drs = draft*rstd ----
        # qn-outer; stats matmuls trail their Wvo group by 2+ groups so the
        # PE never waits on the DVE add / Act square chain, and the qn=0
        # stats chain + rstd broadcast hide under the qn=1 Wvo matmuls.
        s_tiles = {}
        dsls = {}
        sqs = {}

        def wvo_group(qn, i, oc):
            ppt = pp.tile([128, 512], F32, tag="pp", name=f"wv_{oc}_{qn}")
            for kc in range(8):
                nc.tensor.matmul(
                    ppt[:, :],
                    wvq[i][:, kc * 256 + (oc % 2) * 128:
                           kc * 256 + (oc % 2) * 128 + 128],
                    ysb[:, kc * 1024 + qn * 512:kc * 1024 + (qn + 1) * 512],
                    start=(kc == 0), stop=(kc == 7))
            dsl = draft[:, oc * 1024 + qn * 512:oc * 1024 + (qn + 1) * 512]
            nc.vector.tensor_add(
                dsl, ppt[:, :],
                xt[:, oc * XW + WIN + qn * 512:oc * XW + WIN + (qn + 1) * 512])
            sq = sx.tile([128, 512], BF16, tag="sq", name=f"sq_{oc}_{qn}",
                         bufs=4)
            nc.scalar.square(sq[:, :], dsl)
            dsls[(qn, oc)] = dsl
            sqs[(qn, oc)] = sq

        def stat_mms(qn, oc):
            if oc == 0:
                s_tiles[qn] = (
                    sp.tile([1, 512], F32, tag="sp", name=f"s1_{qn}"),
                    sp.tile([1, 512], F32, tag="sp", name=f"s2_{qn}"))
            s1, s2 = s_tiles[qn]
            nc.tensor.matmul(s1[:, :], ones_c, dsls[(qn, oc)],
                             start=(oc == 0), stop=(oc == 7))
            nc.tensor.matmul(s2[:, :], ones_c, sqs[(qn, oc)][:, :],
                             start=(oc == 0), stop=(oc == 7))

        def stats_chain(qn):
            s1, s2 = s_tiles[qn]
            nc.vector.tensor_scalar_mul(s1[:, :], s1[:, :], 1.0 / H)
            mu2 = statr[0:1, qn * 512:(qn + 1) * 512]
            nc.scalar.square(mu2, s1[:, :])
            nc.vector.tensor_scalar_mul(s2[:, :], s2[:, :], 1.0 / H)
            nc.vector.tensor_sub(s2[:, :], s2[:, :], mu2)
            nc.scalar.activation(s2[:, :], s2[:, :], AF.Sqrt, bias=eps_t)
            rstd = statr[0:1, qn * 512:(qn + 1) * 512]
            nc.vector.reciprocal(rstd, s2[:, :])
            with nc.allow_low_precision(reason="bf16 operand copies of stats"):
                nc.vector.tensor_mul(
                    statb[0:1, 1024 + qn * 512:1024 + (qn + 1) * 512],
                    s1[:, :], rstd)
                nc.vector.tensor_copy(statb[0:1, qn * 512:(qn + 1) * 512],
                                      rstd)

        def rb_broadcast(qn):
            rb = pp.tile([128, 512], F32, tag="pp", name=f"rb{qn}")
            nc.tensor.matmul(rb[:, :], ones_r,
                             statb[0:1, qn * 512:(qn + 1) * 512],
                             start=True, stop=True)
            # rb rows are exact copies of bf16 rstd values -> bf16 is lossless
            rbs = sx.tile([128, 512], BF16, tag="rb", name=f"rbs{qn}")
            nc.any.tensor_copy(rbs[:, :], rb[:, :])
            return rbs

        def drs_muls(qn, rbs):
            # Pool-heavy: the DVE queue is busy with residual adds here
            for oc in range(8):
                sl = slice(oc * 1024 + qn * 512, oc * 1024 + qn * 512 + 512)
                eng = nc.vector if oc in (3, 7) else nc.gpsimd
                eng.tensor_mul(drs[:, sl], draft[:, sl], rbs[:, :])

        for i in range(4):
            for oc in (2 * i, 2 * i + 1):
                wvo_group(0, i, oc)
                if oc >= 2:
                    stat_mms(0, oc - 2)

        def mlp1_group(i, mc, qn):
            ppt = pp.tile([128, 512], F32, tag="pp", name=f"m1_{mc}_{qn}")
            for kc in range(8):
                nc.tensor.matmul(
                    ppt[:, :],
                    w1q[i][:, kc * 256 + (mc % 2) * 128:
                           kc * 256 + (mc % 2) * 128 + 128],
                    drs[:, kc * 1024 + qn * 512:kc * 1024 + (qn + 1) * 512],
                    start=(kc == 0), stop=False)
            nc.tensor.matmul(
                ppt[:, :],
                nw1s[0:1, mc * 128:(mc + 1) * 128],
                statb[0:1, 1024 + qn * 512:1024 + (qn + 1) * 512],
                start=False, stop=True)
            nc.scalar.activation(
                h1[:, mc * 1024 + qn * 512:mc * 1024 + (qn + 1) * 512],
                ppt[:, :], AF.Gelu, bias=b1c[:, mc:mc + 1], scale=1.0)

        def mlp2_group(i, oc, qn):
            ppt = pp.tile([128, 512], F32, tag="pp", name=f"m2_{oc}_{qn}")
            for mc in range(4):
                nc.tensor.matmul(
                    ppt[:, :],
                    w2h[i][:, mc * 512 + (oc % 4) * 128:
                           mc * 512 + (oc % 4) * 128 + 128],
                    h1[:, mc * 1024 + qn * 512:mc * 1024 + (qn + 1) * 512],
                    start=(mc == 0), stop=(mc == 3))
            ot = sx.tile([128, 512], BF16, tag="ot", bufs=3,
                         name=f"ot_{oc}_{qn}")
            dsl = draft[:, oc * 1024 + qn * 512:oc * 1024 + (qn + 1) * 512]
            if (oc * 2 + qn) % 3 == 2:
                ppc = sx.tile([128, 512], BF16, tag="ppc", bufs=2,
                              name=f"ppc_{oc}_{qn}")
                nc.scalar.activation(ppc[:, :], ppt[:, :], AF.Identity,
                                     bias=b2c[:, oc:oc + 1])
                nc.gpsimd.tensor_add(ot[:, :], ppc[:, :], dsl)
            else:
                nc.vector.scalar_tensor_tensor(
                    ot[:, :], ppt[:, :], b2c[:, oc:oc + 1], dsl,
                    op0=OP.add, op1=OP.add)
            nc.sync.dma_start(
                outT[oc * 128:(oc + 1) * 128, qn * 512:(qn + 1) * 512],
                ot[:, :])

        # qn=1 Wvo groups, with the qn=0 stats tail / broadcast interleaved
        wvo_group(1, 0, 0)
        stat_mms(0, 6)
        wvo_group(1, 0, 1)
        stat_mms(0, 7)
        stats_chain(0)
        wvo_group(1, 1, 2)
        rbs0 = rb_broadcast(0)
        drs_muls(0, rbs0)
        wvo_group(1, 1, 3)
        stat_mms(1, 0)
        wvo_group(1, 2, 4)
        stat_mms(1, 1)
        wvo_group(1, 2, 5)
        stat_mms(1, 2)
        wvo_group(1, 3, 6)
        stat_mms(1, 3)
        stat_mms(1, 4)
        wvo_group(1, 3, 7)
        stat_mms(1, 5)

        if DEBUG_TAPS:
            nc.sync.dma_start(taps["draft_d"], draft[:, :])

        # MLP1 qn=0 groups cover the qn=1 stats tail + broadcast; MLP2's
        # qn=0 groups (which only need h1 qn=0) run before MLP1 qn=1 so the
        # chain -> rb1 -> drs scaling never stalls the PE.
        mlp1_group(0, 0, 0)
        stat_mms(1, 6)
        stat_mms(1, 7)
        stats_chain(1)
        mlp1_group(0, 1, 0)
        rbs1 = rb_broadcast(1)
        drs_muls(1, rbs1)
        mlp1_group(1, 2, 0)
        mlp1_group(1, 3, 0)
        for i in range(2):
            for oc in range(4 * i, 4 * i + 4):
                mlp2_group(i, oc, 0)
        for i in range(2):
            for mc in (2 * i, 2 * i + 1):
                mlp1_group(i, mc, 1)
        for i in range(2):
            for oc in range(4 * i, 4 * i + 4):
                mlp2_group(i, oc, 1)

        if DEBUG_TAPS:
            nc.sync.dma_start(taps["stat_d"], statr[:, :])
            nc.sync.dma_start(taps["statb_d"], statb[:, :])
            nc.sync.dma_start(taps["drs_d"], drs[:, :])
            nc.sync.dma_start(taps["h1_d"], h1[:, :])

    nc.compile()
    return nc


def _get_nc():
    if "nc" not in _CACHE:
        _CACHE["nc"] = _build()
    return _CACHE["nc"]


def _masks():
    kk = np.arange(KWIN)[None, :]
    p = np.arange(128)[:, None]
    band = (kk - p >= 1) & (kk - p <= WIN)
    mR = np.where(band, 0.0, -1e30).astype(np.float32)
    m_first = np.where(band & (kk >= WIN), 0.0, -1e30).astype(np.float32)
    return m_first, mR


def kernel(hidden_states, Wq, Wk, Wv, Wo, ln_w, ln_b, W1, b1, W2, b2):
    bf16 = ml_dtypes.bfloat16
    hs = np.ascontiguousarray(np.asarray(hidden_states, np.float32))
    Wq, Wk, Wv, Wo = (np.asarray(a, np.float32) for a in (Wq, Wk, Wv, Wo))
    ln_w, ln_b = np.asarray(ln_w, np.float32), np.asarray(ln_b, np.float32)
    W1, b1 = np.asarray(W1, np.float32), np.asarray(b1, np.float32)
    W2, b2 = np.asarray(W2, np.float32), np.asarray(b2, np.float32)

    nc = _get_nc()
    m_first, mR = _masks()
    A = (Wq.T @ Wk) / np.sqrt(H)
    Wvo = Wo @ Wv
    w1T = W1.T * ln_w[:, None]
    cbm = np.zeros((128, 769), np.float32)
    cbm[:, 0:128] = np.eye(128, dtype=np.float32)
    cbm[:, 128] = 1.0
    cbm[0, 129:257] = 1.0
    cbm[0, 257:769] = -w1T.sum(0)

    def cf_pack(m0):
        cfm = np.zeros((128, 333), np.float32)
        cfm[:, 0:KWIN] = m0
        cfm[:, KWIN:2 * KWIN] = mR
        cfm[:, 320:324] = (b1 + W1 @ ln_b).reshape(4, 128).T
        cfm[:, 324:332] = b2.reshape(8, 128).T
        cfm[0, 332] = 1e-5
        return cfm
    cf_first, cf_rest = cf_pack(m_first), cf_pack(mR)
    cm_first = np.concatenate([m_first, mR], 1).astype(bf16)
    cm_rest = np.concatenate([mR, mR], 1).astype(bf16)
    shared = {
        "cb": cbm.astype(bf16),
        "a": np.ascontiguousarray(A).astype(bf16),
        "wvo": np.ascontiguousarray(Wvo.T).astype(bf16),
        "w1": np.ascontiguousarray(w1T).astype(bf16),
        "w2": np.ascontiguousarray(W2.T).astype(bf16),
    }
    in_maps = []
    for c in range(N_CORES):
        b, ch = divmod(c, 4)
        rows = hs[b, ch * SL:(ch + 1) * SL]
        halo = (np.zeros((WIN, H), np.float32) if ch == 0
                else hs[b, ch * SL - WIN:ch * SL])
        xfull = np.concatenate([halo, rows], 0)          # [1056, H]
        xn = np.zeros((9 * 128, H), np.float32)
        xn[0:XW] = xfull
        m = dict(shared)
        m["xT"] = np.ascontiguousarray(xfull.T).astype(bf16)
        m["xn"] = xn.astype(bf16)
        m["cf"] = cf_first if ch == 0 else cf_rest
        m["cm"] = cm_first if ch == 0 else cm_rest
        in_maps.append(m)

    res = run_bass_kernel_spmd(nc, in_maps, list(range(N_CORES)))
    _CACHE["res"] = res
    out = np.empty((B, S, H), np.float32)
    for c in range(N_CORES):
        b, ch = divmod(c, 4)
        out[b, ch * SL:(ch + 1) * SL] = \
            res.results[c]["outT"].T.astype(np.float32)
    return out
